# revision 2
# baseline (speedup 1.0000x reference)
"""Trainium2 Bass kernel for nn_BusEmbedding (moe_routing).

Computes out[t] = tanh(feat[t] @ W_e + b_e) where e is selected per-row by
bus_type[t] in {1,2,3} (rows with other values stay zero).

Strategy (pure data parallel over 8 cores, N sharded, weights replicated):
  The routing is folded into a single K=128 matmul per 1024 tokens.
  For each 128-token group we build 16 "slots":
    slot 2e+k = feat[t,k] * (bus_type[t]==e+1)   (k in {0,1}, e in {0,1,2})
    slot 6+e  = (bus_type[t]==e+1)
    slots 9..15 = 0
  Eight groups' slots are packed into a [128 token, 128 slot] tile, PE-transposed,
  and multiplied against a block-diagonal [128, 1024] weight matrix whose g-th
  column block holds [W_e rows; b_e] for the g-th group's slots. The result is
  the *selected* expert's pre-activation for 8 groups at once; one Tanh
  activation pass and a strided DMA write follow.
"""

import sys
from contextlib import ExitStack

import numpy as np

sys.path.insert(0, "/opt/trn_rl_repo")

import concourse.bass as bass  # noqa: E402
import concourse.bacc as bacc  # noqa: E402
import concourse.mybir as mybir  # noqa: E402
import concourse.tile as tile  # noqa: E402
from concourse.bass_utils import run_bass_kernel_spmd  # noqa: E402

FP = mybir.dt.float32
D = 128
SUPER = 16384  # tokens per supertile (128 partitions x 128)
N_CORES = 8
PER_CORE = 131072  # padded tokens per core (8 supertiles)
N_FULL = 1_000_000

_NC_CACHE = {}


def _body(ctx, tc, out, feat, btf, wbig, ident, n_tokens):
    nc = tc.nc
    n_super = n_tokens // SUPER

    const_pool = ctx.enter_context(tc.tile_pool(name="const", bufs=1))
    wbig_sb = const_pool.tile([128, 1024], FP)
    nc.sync.dma_start(wbig_sb[:], wbig)
    ident_sb = const_pool.tile([128, 128], FP)
    nc.sync.dma_start(ident_sb[:], ident)

    in_pool = ctx.enter_context(tc.tile_pool(name="inp", bufs=3))
    tp_ps = ctx.enter_context(tc.tile_pool(name="tp_ps", bufs=3, space="PSUM"))
    tsb_pool = ctx.enter_context(tc.tile_pool(name="tsb", bufs=2))
    pk_pool = ctx.enter_context(tc.tile_pool(name="pk", bufs=2))
    xsb_pool = ctx.enter_context(tc.tile_pool(name="xsb", bufs=3))
    mm_pool = ctx.enter_context(tc.tile_pool(name="mm", bufs=2, space="PSUM"))
    out_pool = ctx.enter_context(tc.tile_pool(name="outp", bufs=3))

    feat_v = feat.rearrange("(s p j) k -> s p (j k)", p=128, j=128)  # [S,128,256]
    btf_v = btf.rearrange("(s p j) -> s p j", p=128, j=128)  # [S,128,128]
    out_v = out.rearrange("(q g p) d -> q p g d", g=32, p=128)  # [4S,128,32,128]

    for s in range(n_super):
        f_c = in_pool.tile([128, 256], FP, tag="fc")
        nc.sync.dma_start(f_c[:], feat_v[s])
        bt_c = in_pool.tile([128, 128], FP, tag="btc")
        nc.sync.dma_start(bt_c[:], btf_v[s])

        btT_ps = tp_ps.tile([128, 128], FP, tag="tp")
        nc.tensor.transpose(btT_ps[:], bt_c[:], ident_sb[:])
        btT = tsb_pool.tile([128, 128], FP, tag="btT")
        nc.vector.tensor_copy(btT[:], btT_ps[:])

        f0T_ps = tp_ps.tile([128, 128], FP, tag="tp")
        nc.tensor.transpose(f0T_ps[:], f_c[:, 0::2], ident_sb[:])
        f0T = tsb_pool.tile([128, 128], FP, tag="f0T")
        nc.vector.tensor_copy(f0T[:], f0T_ps[:])

        f1T_ps = tp_ps.tile([128, 128], FP, tag="tp")
        nc.tensor.transpose(f1T_ps[:], f_c[:, 1::2], ident_sb[:])
        f1T = tsb_pool.tile([128, 128], FP, tag="f1T")
        nc.vector.tensor_copy(f1T[:], f1T_ps[:])

        P = pk_pool.tile([128, 2048], FP)
        P4 = P.rearrange("p (a b c) -> p a b c", a=16, b=8, c=16)
        btT3 = btT.rearrange("p (a b) -> p a b", a=16, b=8)
        f0T3 = f0T.rearrange("p (a b) -> p a b", a=16, b=8)
        f1T3 = f1T.rearrange("p (a b) -> p a b", a=16, b=8)
        nc.vector.memset(P4[:, :, :, 9:16], 0.0)
        for e in (1, 2, 3):
            ei = e - 1
            nc.vector.tensor_scalar(
                P4[:, :, :, 6 + ei], btT3[:], float(e), None,
                op0=mybir.AluOpType.is_equal,
            )
            nc.vector.scalar_tensor_tensor(
                P4[:, :, :, 2 * ei], btT3[:], float(e), f0T3[:],
                op0=mybir.AluOpType.is_equal, op1=mybir.AluOpType.mult,
            )
            nc.vector.scalar_tensor_tensor(
                P4[:, :, :, 2 * ei + 1], btT3[:], float(e), f1T3[:],
                op0=mybir.AluOpType.is_equal, op1=mybir.AluOpType.mult,
            )

        for blk in range(4):
            ob = out_pool.tile([128, 4096], FP)
            for q in range(4):
                pt = blk * 4 + q
                xps = tp_ps.tile([128, 128], FP, tag="tp")
                nc.tensor.transpose(xps[:], P[:, pt * 128:(pt + 1) * 128], ident_sb[:])
                xsb = xsb_pool.tile([128, 128], FP)
                nc.vector.tensor_copy(xsb[:], xps[:])
                mm = mm_pool.tile([128, 1024], FP)
                nc.tensor.matmul(mm[:, 0:512], xsb[:], wbig_sb[:, 0:512],
                                 start=True, stop=True)
                nc.tensor.matmul(mm[:, 512:1024], xsb[:], wbig_sb[:, 512:1024],
                                 start=True, stop=True)
                nc.scalar.activation(ob[:, q * 1024:(q + 1) * 1024], mm[:],
                                     mybir.ActivationFunctionType.Tanh)
            nc.sync.dma_start(out_v[s * 4 + blk],
                              ob.rearrange("p (g d) -> p g d", g=32))


def build_nc(n_tokens=PER_CORE):
    key = n_tokens
    if key in _NC_CACHE:
        return _NC_CACHE[key]
    nc = bacc.Bacc("TRN2", target_bir_lowering=False, debug=False)
    feat = nc.dram_tensor("feat", [n_tokens, 2], FP, kind="ExternalInput").ap()
    btf = nc.dram_tensor("btf", [n_tokens], FP, kind="ExternalInput").ap()
    wbig = nc.dram_tensor("wbig", [128, 1024], FP, kind="ExternalInput").ap()
    ident = nc.dram_tensor("ident", [128, 128], FP, kind="ExternalInput").ap()
    out = nc.dram_tensor("out", [n_tokens, D], FP, kind="ExternalOutput").ap()
    with tile.TileContext(nc) as tc:
        with ExitStack() as ctx:
            _body(ctx, tc, out, feat, btf, wbig, ident, n_tokens)
    nc.compile()
    _NC_CACHE[key] = nc
    return nc


def make_wbig(W_slack, b_slack, W_gen, b_gen, W_load, b_load):
    W_list = [np.asarray(w, np.float32) for w in (W_slack, W_gen, W_load)]
    b_list = [np.asarray(b, np.float32) for b in (b_slack, b_gen, b_load)]
    WBig = np.zeros((128, 1024), np.float32)
    for g in range(8):
        col = g * 128
        for ei in range(3):
            WBig[g * 16 + 2 * ei + 0, col:col + 128] = W_list[ei][0]
            WBig[g * 16 + 2 * ei + 1, col:col + 128] = W_list[ei][1]
            WBig[g * 16 + 6 + ei, col:col + 128] = b_list[ei]
    return WBig


def kernel(feat, bus_type, W_slack, b_slack, W_gen, b_gen, W_load, b_load,
           **run_kwargs):
    feat = np.ascontiguousarray(np.asarray(feat, np.float32))
    bt = np.asarray(bus_type)
    n = feat.shape[0]
    npad = N_CORES * PER_CORE
    assert n <= npad

    featp = np.zeros((npad, 2), np.float32)
    featp[:n] = feat
    btp = np.zeros((npad,), np.float32)
    btp[:n] = bt.astype(np.float32)  # values are tiny ints; cast is exact
    wbig = make_wbig(W_slack, b_slack, W_gen, b_gen, W_load, b_load)
    ident = np.eye(128, dtype=np.float32)

    nc = build_nc(PER_CORE)
    in_maps = [
        {
            "feat": featp[i * PER_CORE:(i + 1) * PER_CORE],
            "btf": btp[i * PER_CORE:(i + 1) * PER_CORE],
            "wbig": wbig,
            "ident": ident,
        }
        for i in range(N_CORES)
    ]
    res = run_bass_kernel_spmd(nc, in_maps, list(range(N_CORES)), **run_kwargs)
    out = np.concatenate([res.results[i]["out"] for i in range(N_CORES)], axis=0)
    kernel.last_result = res
    return out[:n]


# revision 4
# speedup vs baseline: 1.0973x; 1.0973x over previous
"""Trainium2 Bass kernel for nn_BusEmbedding (moe_routing).

Computes out[t] = tanh(feat[t] @ W_e + b_e) where e is selected per-row by
bus_type[t] in {1,2,3} (rows with other values stay zero).

Strategy (pure data parallel over 8 cores, N sharded, weights replicated):
  The routing is folded into a single K=128 matmul per 1024 tokens.
  For each 128-token group we build 16 "slots":
    slot 2e+k = feat[t,k] * (bus_type[t]==e+1)   (k in {0,1}, e in {0,1,2})
    slot 6+e  = (bus_type[t]==e+1)
    slots 9..15 = 0
  Eight groups' slots are packed into a [128 token, 128 slot] tile, PE-transposed,
  and multiplied against a block-diagonal [128, 1024] weight matrix whose g-th
  column block holds [W_e rows; b_e] for the g-th group's slots. The result is
  the *selected* expert's pre-activation for 8 groups at once; one Tanh
  activation pass and a strided DMA write follow.
"""

import sys
from contextlib import ExitStack

import numpy as np

sys.path.insert(0, "/opt/trn_rl_repo")

import concourse.bass as bass  # noqa: E402
import concourse.bacc as bacc  # noqa: E402
import concourse.mybir as mybir  # noqa: E402
import concourse.tile as tile  # noqa: E402
from concourse.bass_utils import run_bass_kernel_spmd  # noqa: E402

FP = mybir.dt.float32
FPR = mybir.dt.float32r
D = 128
SUPER = 16384  # tokens per supertile (128 partitions x 128)
N_CORES = 8
PER_CORE = 131072  # padded tokens per core (8 supertiles)
N_FULL = 1_000_000

_NC_CACHE = {}


def _body(ctx, tc, out, feat, btf, wbig, ident, n_tokens):
    nc = tc.nc
    n_super = n_tokens // SUPER

    const_pool = ctx.enter_context(tc.tile_pool(name="const", bufs=1))
    wbig_sb = const_pool.tile([128, 1024], FP)
    nc.sync.dma_start(wbig_sb[:], wbig)
    ident_sb = const_pool.tile([128, 128], FP)
    nc.sync.dma_start(ident_sb[:], ident)
    wbig_r = const_pool.tile([128, 1024], FPR)
    nc.vector.tensor_copy(wbig_r[:], wbig_sb[:])

    in_pool = ctx.enter_context(tc.tile_pool(name="inp", bufs=3))
    tp_ps = ctx.enter_context(tc.tile_pool(name="tp_ps", bufs=2, space="PSUM"))
    tsb_pool = ctx.enter_context(tc.tile_pool(name="tsb", bufs=2))
    pk_pool = ctx.enter_context(tc.tile_pool(name="pk", bufs=2))
    xsb_pool = ctx.enter_context(tc.tile_pool(name="xsb", bufs=3))
    mm_pool = ctx.enter_context(tc.tile_pool(name="mm", bufs=3, space="PSUM"))
    out_pool = ctx.enter_context(tc.tile_pool(name="outp", bufs=3))

    feat_v = feat.rearrange("(s p j) k -> s p (j k)", p=128, j=128)  # [S,128,256]
    btf_v = btf.rearrange("(s p j) -> s p j", p=128, j=128)  # [S,128,128]
    out_v = out.rearrange("(q g p) d -> q p g d", g=32, p=128)  # [4S,128,32,128]

    for s in range(n_super):
        f_c = in_pool.tile([128, 256], FP, tag="fc")
        nc.sync.dma_start(f_c[:], feat_v[s])
        bt_c = in_pool.tile([128, 128], FP, tag="btc")
        nc.sync.dma_start(bt_c[:], btf_v[s])

        btT_ps = tp_ps.tile([128, 128], FP, tag="tp")
        nc.tensor.transpose(btT_ps[:], bt_c[:], ident_sb[:])
        btT = tsb_pool.tile([128, 128], FP, tag="btT")
        nc.vector.tensor_copy(btT[:], btT_ps[:])

        f0T_ps = tp_ps.tile([128, 128], FP, tag="tp")
        nc.tensor.transpose(f0T_ps[:], f_c[:, 0::2], ident_sb[:])
        f0T = tsb_pool.tile([128, 128], FP, tag="f0T")
        nc.vector.tensor_copy(f0T[:], f0T_ps[:])

        f1T_ps = tp_ps.tile([128, 128], FP, tag="tp")
        nc.tensor.transpose(f1T_ps[:], f_c[:, 1::2], ident_sb[:])
        f1T = tsb_pool.tile([128, 128], FP, tag="f1T")
        nc.vector.tensor_copy(f1T[:], f1T_ps[:])

        P = pk_pool.tile([128, 2048], FP)
        P4 = P.rearrange("p (a b c) -> p a b c", a=16, b=8, c=16)
        btT3 = btT.rearrange("p (a b) -> p a b", a=16, b=8)
        f0T3 = f0T.rearrange("p (a b) -> p a b", a=16, b=8)
        f1T3 = f1T.rearrange("p (a b) -> p a b", a=16, b=8)
        nc.vector.memset(P4[:, :, :, 9:16], 0.0)
        for e in (1, 2, 3):
            ei = e - 1
            nc.vector.tensor_scalar(
                P4[:, :, :, 6 + ei], btT3[:], float(e), None,
                op0=mybir.AluOpType.is_equal,
            )
            nc.vector.scalar_tensor_tensor(
                P4[:, :, :, 2 * ei], btT3[:], float(e), f0T3[:],
                op0=mybir.AluOpType.is_equal, op1=mybir.AluOpType.mult,
            )
            nc.vector.scalar_tensor_tensor(
                P4[:, :, :, 2 * ei + 1], btT3[:], float(e), f1T3[:],
                op0=mybir.AluOpType.is_equal, op1=mybir.AluOpType.mult,
            )

        for blk in range(4):
            ob = out_pool.tile([128, 4096], FP)
            for q in range(4):
                pt = blk * 4 + q
                xps = tp_ps.tile([128, 128], FP, tag="tp")
                nc.tensor.transpose(xps[:], P[:, pt * 128:(pt + 1) * 128], ident_sb[:])
                xsb = xsb_pool.tile([128, 128], FPR)
                nc.vector.tensor_copy(xsb[:], xps[:])
                mm = mm_pool.tile([128, 1024], FP)
                nc.tensor.matmul(mm[:, 0:512], xsb[:], wbig_r[:, 0:512],
                                 start=True, stop=True)
                nc.tensor.matmul(mm[:, 512:1024], xsb[:], wbig_r[:, 512:1024],
                                 start=True, stop=True)
                nc.scalar.activation(ob[:, q * 1024:(q + 1) * 1024], mm[:],
                                     mybir.ActivationFunctionType.Tanh)
            nc.sync.dma_start(out_v[s * 4 + blk],
                              ob.rearrange("p (g d) -> p g d", g=32))


def build_nc(n_tokens=PER_CORE):
    key = n_tokens
    if key in _NC_CACHE:
        return _NC_CACHE[key]
    nc = bacc.Bacc("TRN2", target_bir_lowering=False, debug=False)
    feat = nc.dram_tensor("feat", [n_tokens, 2], FP, kind="ExternalInput").ap()
    btf = nc.dram_tensor("btf", [n_tokens], FP, kind="ExternalInput").ap()
    wbig = nc.dram_tensor("wbig", [128, 1024], FP, kind="ExternalInput").ap()
    ident = nc.dram_tensor("ident", [128, 128], FP, kind="ExternalInput").ap()
    out = nc.dram_tensor("out", [n_tokens, D], FP, kind="ExternalOutput").ap()
    with tile.TileContext(nc) as tc:
        with ExitStack() as ctx:
            _body(ctx, tc, out, feat, btf, wbig, ident, n_tokens)
    nc.compile()
    _NC_CACHE[key] = nc
    return nc


def make_wbig(W_slack, b_slack, W_gen, b_gen, W_load, b_load):
    W_list = [np.asarray(w, np.float32) for w in (W_slack, W_gen, W_load)]
    b_list = [np.asarray(b, np.float32) for b in (b_slack, b_gen, b_load)]
    WBig = np.zeros((128, 1024), np.float32)
    for g in range(8):
        col = g * 128
        for ei in range(3):
            WBig[g * 16 + 2 * ei + 0, col:col + 128] = W_list[ei][0]
            WBig[g * 16 + 2 * ei + 1, col:col + 128] = W_list[ei][1]
            WBig[g * 16 + 6 + ei, col:col + 128] = b_list[ei]
    return WBig


def kernel(feat, bus_type, W_slack, b_slack, W_gen, b_gen, W_load, b_load,
           **run_kwargs):
    feat = np.ascontiguousarray(np.asarray(feat, np.float32))
    bt = np.asarray(bus_type)
    n = feat.shape[0]
    npad = N_CORES * PER_CORE
    assert n <= npad

    featp = np.zeros((npad, 2), np.float32)
    featp[:n] = feat
    btp = np.zeros((npad,), np.float32)
    btp[:n] = bt.astype(np.float32)  # values are tiny ints; cast is exact
    wbig = make_wbig(W_slack, b_slack, W_gen, b_gen, W_load, b_load)
    ident = np.eye(128, dtype=np.float32)

    nc = build_nc(PER_CORE)
    in_maps = [
        {
            "feat": featp[i * PER_CORE:(i + 1) * PER_CORE],
            "btf": btp[i * PER_CORE:(i + 1) * PER_CORE],
            "wbig": wbig,
            "ident": ident,
        }
        for i in range(N_CORES)
    ]
    res = run_bass_kernel_spmd(nc, in_maps, list(range(N_CORES)), **run_kwargs)
    out = np.concatenate([res.results[i]["out"] for i in range(N_CORES)], axis=0)
    kernel.last_result = res
    return out[:n]
